# revision 1
# baseline (speedup 1.0000x reference)
"""Causal self-attention (B=4, S=2048, D=768, H=12) on 8 trn2 NeuronCores.

Sharding (Megatron-style): DP over the 4 batches x TP=2 over heads.
Core c handles batch c//2 with heads (c%2)*6 .. +6: qkv_proj column-parallel,
out_proj row-parallel; the TP pair's partial outputs are summed on the host.

Per-core kernel (all matmuls fp32r, fp32 data widths):
  A. stream x [2048,768], PE-transpose to xT [d(part), s]
  B. qkT = (x @ Wqk)^T directly in [feat(part), s] layout (W stationary,
     xT moving); V in natural [s(part), feat] layout with a ones column
     appended (V') so the PV matmul also produces the softmax denominator.
     Score scale 1/sqrt(64) and qkv bias are folded in (scale on host into
     Wq/bq; bias added during the PSUM->SBUF copy, per-partition in the
     transposed layout).
  C. flash-style causal attention per head: S^T tile = K_tile @ Q^T
     (contraction = head dim 64), exp on ACT batched 2 k-tiles per
     ACTIVATE (amortizes the 352-cycle fixed cost; gap columns of
     diagonal groups hold garbage exp values that no PV matmul reads),
     causal via narrowed matmuls + one 128x128 mask multiply (GPSIMD)
     per diagonal block; O^T accumulated in PSUM over k tiles via
     lhsT=V' (no max subtraction: scores are O(5), fp32 exp is safe);
     denominator row broadcast across partitions with a K=1 ones matmul,
     applied on the PSUM->SBUF copy.
  D. out_partial = O @ Wout_slice via lhsT=OT chunks, written [s, 768].
"""
import numpy as np
import concourse.bass as bass
import concourse.mybir as mybir
import concourse.tile as tile
from concourse import bacc
from concourse.bass_utils import run_bass_kernel_spmd
from concourse.masks import make_identity

B, S, D = 4, 2048, 768
H, HD = 12, 64
N_CORES = 8
HPC = H // 2          # heads per core = 6
FQK = HPC * HD        # 384 features per core for each of q,k,v
F32 = mybir.dt.float32
F32R = mybir.dt.float32r

N_ST = S // 128       # 16 s tiles
N_QC = S // 512       # 4 q chunks
N_DT = D // 128       # 6 d_model tiles

TRACE = False         # set by test.py for profiling runs
_CACHE = {}
PHASE_MARKS = []      # (phase_name, first_inst_id) — filled during _emit


def _mark(nc, name):
    PHASE_MARKS.append((name, nc.next_id()))


def _emit(nc):
    xt_d = nc.dram_tensor("xt", [D, S], F32R, kind="ExternalInput").ap()
    wqkv_d = nc.dram_tensor("wqkv", [D, 3 * FQK], F32R, kind="ExternalInput").ap()
    bqk_d = nc.dram_tensor("bqk", [128, 6], F32, kind="ExternalInput").ap()
    vb_d = nc.dram_tensor("vb", [128, FQK], F32, kind="ExternalInput").ap()
    wout_d = nc.dram_tensor("wout", [FQK, D], F32R, kind="ExternalInput").ap()
    out_d = nc.dram_tensor("out", [S, D], F32, kind="ExternalOutput").ap()

    with tile.TileContext(nc) as tc:
        with tc.tile_pool(name="const", bufs=1) as pc, \
             tc.tile_pool(name="qkT", bufs=1) as pqk, \
             tc.tile_pool(name="vn", bufs=1) as pvn, \
             tc.tile_pool(name="wstr", bufs=3) as pw, \
             tc.tile_pool(name="pt", bufs=4) as ppt, \
             tc.tile_pool(name="ep", bufs=2) as pep, \
             tc.tile_pool(name="oraw", bufs=8) as por, \
             tc.tile_pool(name="outp", bufs=2) as pout, \
             tc.tile_pool(name="ps", bufs=3, space="PSUM") as pp, \
             tc.tile_pool(name="psd", bufs=1, space="PSUM") as ppd, \
             tc.tile_pool(name="pso", bufs=1, space="PSUM") as ppo:

            # causal block mask: keep where local q (free) >= local k (part)
            mask = pc.tile([128, 128], F32)
            nc.gpsimd.memset(mask[:], 1.0)
            nc.gpsimd.affine_select(
                out=mask[:], in_=mask[:], compare_op=mybir.AluOpType.is_ge,
                fill=0.0, base=0, channel_multiplier=-1, pattern=[[1, 128]])
            bqk_sb = pc.tile([128, 6], F32)
            nc.sync.dma_start(bqk_sb[:], bqk_d[:])
            vb_sb = pc.tile([128, FQK], F32)
            nc.sync.dma_start(vb_sb[:], vb_d[:])
            wv_sb = pc.tile([128, N_DT, FQK], F32R)
            wout_sb = pc.tile([128, FQK // 128, D], F32R)

            # Vn: [s(part), s_tile, head, 65] with ones col at 64
            vn = pvn.tile([128, N_ST, HPC, HD + 1], F32R)
            # qkT: [feat%128(part), f_tile (0-2 q | 3-5 k), s]
            qkT = pqk.tile([128, 6, S], F32R)

            # ---- attention k-loop for one (head, q-chunk), software-
            # pipelined across jobs: each group's PV matmuls are deferred
            # until after the NEXT group's S^T+exp are emitted, and the
            # job's final PV pair is deferred into the next job (via the
            # `pending` closure) so the next job's first S^T reaches PE
            # before it — keeping ACT fed across job boundaries. Returns
            # (state, tail): state["oraw"] is filled once tail() has been
            # emitted; normalization reads it afterwards. ----
            _pend = {"t": None}

            def attn_kloop(h, qc):
                po = (h % 2) * 64
                qt = h // 2         # q f_tile
                kt_f = 3 + h // 2   # k f_tile
                ps_o = ppo.tile([128, 512], F32, tag="o")
                n_kt = 4 * (qc + 1)
                state = {}

                def flush(ktg, offs, pt, last):
                    for j in range(2):
                        kt = ktg + j
                        q_off = offs[j]
                        if kt * 128 >= qc * 512:  # diagonal block
                            sl = slice(j * 512 + q_off, j * 512 + q_off + 128)
                            nc.gpsimd.tensor_tensor(
                                pt[:, sl], pt[:, sl], mask[:],
                                mybir.AluOpType.mult)
                        nc.tensor.matmul(
                            ps_o[0:HD + 1, q_off:], vn[:, kt, h, :],
                            pt[:, j * 512 + q_off:(j + 1) * 512],
                            start=(kt == 0), stop=(kt == n_kt - 1))
                    if last:
                        oraw = por.tile([65, 512], F32, tag="oraw")
                        nc.vector.tensor_copy(oraw[:], ps_o[0:HD + 1, :])
                        state["oraw"] = oraw

                prev_grp = None
                for ktg in range(0, n_kt, 2):
                    ps_s = pp.tile([128, 1024], F32, tag="s")
                    offs = []
                    for j in range(2):
                        kt = ktg + j
                        q_off = max(0, kt * 128 - qc * 512)
                        offs.append(q_off)
                        nc.tensor.matmul(
                            ps_s[:, j * 512 + q_off:(j + 1) * 512],
                            qkT[po:po + 64, kt_f, kt * 128:(kt + 1) * 128],
                            qkT[po:po + 64, qt,
                                qc * 512 + q_off:(qc + 1) * 512],
                            start=True, stop=True)
                    pt = ppt.tile([128, 1024], F32R, tag="pt")
                    nc.scalar.activation(
                        pt[:, offs[0]:], ps_s[:, offs[0]:],
                        mybir.ActivationFunctionType.Exp)
                    if ktg == 0 and _pend["t"] is not None:
                        _pend["t"]()  # prev job's final PVs after our S^T
                    if prev_grp is not None:
                        flush(*prev_grp, last=False)
                    prev_grp = (ktg, offs, pt)
                _pend["t"] = lambda: flush(*prev_grp, last=True)
                return state

            with tc.tile_pool(name="xT", bufs=1) as pxt:
                xT = pxt.tile([128, N_DT, S], F32R)

                _mark(nc, "A:load")
                # x arrives pre-transposed from the host: DMA straight into
                # xT, s-chunk-major (6 d-chunks per 512-wide s range) so V
                # and the first qkT f_tiles start as soon as chunk 0 lands
                nc.sync.dma_start(
                    wv_sb[:],
                    wqkv_d[:, 2 * FQK:].rearrange("(t p) f -> p t f", p=128))
                for sc in range(N_QC):
                    for dc in range(N_DT):
                        nc.sync.dma_start(
                            xT[:, dc, sc * 512:(sc + 1) * 512],
                            xt_d[dc * 128:(dc + 1) * 128,
                                 sc * 512:(sc + 1) * 512])

                nc.sync.dma_start(
                    wout_sb[:], wout_d.rearrange("(t p) o -> p t o", p=128))

                _mark(nc, "B:qkv")
                # ---- phase B, dependency-ordered so qc0 attention starts
                # as early as possible: V(st 0-3) + f_tiles (0,3) first ->
                # heads 0,1; then the rest of V; then remaining ft pairs ----
                nc.vector.memset(vn[:].bitcast(F32), 1.0)
                vb_h = vb_sb.rearrange("p (h d) -> p h d", d=HD)

                def emit_v(st2):
                    ps_v = pp.tile([128, 1024], F32, tag="s")
                    for j in range(2):
                        st = 2 * st2 + j
                        for dc in range(N_DT):
                            nc.tensor.matmul(
                                ps_v[:, j * 512:j * 512 + FQK],
                                xT[:, dc, st * 128:(st + 1) * 128],
                                wv_sb[:, dc, :],
                                start=(dc == 0), stop=(dc == N_DT - 1))
                    for j in range(2):
                        st = 2 * st2 + j
                        nc.vector.tensor_tensor(
                            vn[:, st, :, 0:HD],
                            ps_v[:, j * 512:j * 512 + FQK].rearrange(
                                "p (h d) -> p h d", d=HD),
                            vb_h, mybir.AluOpType.add)

                def emit_ft(ft):
                    w_t = pw.tile([128, N_DT, 128], F32R, tag="w")
                    nc.sync.dma_start(
                        w_t[:],
                        wqkv_d[:, ft * 128:(ft + 1) * 128].rearrange(
                            "(t p) f -> p t f", p=128))
                    for sc2 in range(2):
                        ps_qk = pp.tile([128, 1024], F32, tag="s")
                        for j in range(2):
                            sc = 2 * sc2 + j
                            for dc in range(N_DT):
                                nc.tensor.matmul(
                                    ps_qk[:, j * 512:(j + 1) * 512],
                                    w_t[:, dc, :],
                                    xT[:, dc, sc * 512:(sc + 1) * 512],
                                    start=(dc == 0),
                                    stop=(dc == N_DT - 1))
                        nc.scalar.activation(
                            qkT[:, ft, sc2 * 1024:(sc2 + 1) * 1024],
                            ps_qk[:],
                            mybir.ActivationFunctionType.Identity,
                            bias=bqk_sb[:, ft:ft + 1])

                oraw0 = {}
                for st2 in range(N_ST // 2):
                    emit_v(st2)
                for pi in range(3):
                    emit_ft(pi)
                    emit_ft(3 + pi)
                    oraw0[2 * pi] = attn_kloop(2 * pi, 0)
                    oraw0[2 * pi + 1] = attn_kloop(2 * pi + 1, 0)

            # xT pool closed; OT reuses its space
            with tc.tile_pool(name="OT", bufs=1) as pot:
                oT = pot.tile([128, FQK // 128, S], F32R)

                def normalize(h, qc, ostate):
                    oraw = ostate["oraw"]
                    po = (h % 2) * 64
                    recip = pep.tile([1, 512], F32R, tag="recip")
                    with nc.allow_low_precision(reason="fp32-width recip"):
                        nc.vector.reciprocal(recip[:], oraw[HD:HD + 1, :])
                    rb = pep.tile([64, 512], F32R, tag="rb")
                    nc.gpsimd.partition_broadcast(rb[:], recip[:])
                    nc.vector.tensor_tensor(
                        oT[po:po + 64, h // 2, qc * 512:(qc + 1) * 512],
                        oraw[0:HD, :], rb[:], mybir.AluOpType.mult)

                def emit_proj(st, tail=False):
                    o_sb = pout.tile([128, D], F32, tag="o_sb")
                    for oc in range(2):
                        # during attention, "d" (1 bank) keeps projs off the
                        # S^T pipeline's "s" slots; in the tail "s" is free
                        # and gives 3-deep rotation instead
                        if tail:
                            ps_big = pp.tile([128, 1024], F32, tag="s")
                            ps_d = ps_big[:, oc * 512:(oc + 1) * 512]
                        else:
                            ps_d = ppd.tile([128, 512], F32, tag="d")
                        for ht in range(FQK // 128):
                            nc.tensor.matmul(
                                ps_d[:, :384],
                                oT[:, ht, st * 128:(st + 1) * 128],
                                wout_sb[:, ht, oc * 384:(oc + 1) * 384],
                                start=(ht == 0),
                                stop=(ht == FQK // 128 - 1))
                        nc.vector.tensor_copy(
                            o_sb[:, oc * 384:(oc + 1) * 384], ps_d[:, :384])
                    nc.sync.dma_start(
                        out_d[st * 128:(st + 1) * 128, :], o_sb[:])

                # previous q-chunk's normalization + out-proj interleave
                # into the next q-chunk's head loop (progressive slot reuse)
                prev = {0: oraw0}
                for qc in range(1, N_QC):
                    _mark(nc, f"C:attn qc={qc}")
                    cur = {}
                    po_ = prev[qc - 1]
                    for h in range(HPC):
                        if h < 3:
                            normalize(2 * h, qc - 1, po_[2 * h])
                            normalize(2 * h + 1, qc - 1, po_[2 * h + 1])
                        else:
                            emit_proj((qc - 1) * 4 + (h - 3))
                        if qc == N_QC - 1 and h >= 2:
                            # last q-chunk: normalize its own early heads
                            # as soon as their k-loops have drained
                            normalize(h - 2, qc, cur[h - 2])
                        cur[h] = attn_kloop(h, qc)
                    emit_proj((qc - 1) * 4 + 3)
                    prev[qc] = cur

                _mark(nc, "D:tail")
                _pend["t"]()  # final job's deferred PV pair + oraw copy
                _pend["t"] = None
                for h in range(HPC - 2, HPC):
                    normalize(h, N_QC - 1, prev[N_QC - 1][h])
                for st in range(12, 16):
                    emit_proj(st, tail=True)


def _build():
    if "nc" not in _CACHE:
        nc = bacc.Bacc("TRN2", target_bir_lowering=False, debug=False,
                       num_devices=N_CORES)
        _emit(nc)
        nc.compile()
        _CACHE["nc"] = nc
    return _CACHE["nc"]


def kernel(x, qkv_w, qkv_b, out_w, out_b):
    x = np.ascontiguousarray(np.asarray(x, dtype=np.float32))
    qkv_w = np.asarray(qkv_w, dtype=np.float32)
    qkv_b = np.asarray(qkv_b, dtype=np.float32)
    out_w = np.asarray(out_w, dtype=np.float32)
    out_b = np.asarray(out_b, dtype=np.float32)

    nc = _build()
    scale = HD ** -0.5
    in_maps = []
    for c in range(N_CORES):
        b, half = c // 2, c % 2
        fq = slice(half * FQK, (half + 1) * FQK)
        fk = slice(D + half * FQK, D + (half + 1) * FQK)
        fv = slice(2 * D + half * FQK, 2 * D + (half + 1) * FQK)
        wq = qkv_w[:, fq] * scale
        wk = qkv_w[:, fk]
        wv = qkv_w[:, fv]
        wqkv = np.ascontiguousarray(
            np.concatenate([wq, wk, wv], axis=1), dtype=np.float32)
        bqk = np.concatenate([qkv_b[fq] * scale, qkv_b[fk]])  # [768]
        bqk = np.ascontiguousarray(
            bqk.reshape(6, 128).T, dtype=np.float32)          # [128, 6]
        vb = np.ascontiguousarray(
            np.broadcast_to(qkv_b[fv], (128, FQK)), dtype=np.float32)
        wout = np.ascontiguousarray(
            out_w[half * FQK:(half + 1) * FQK, :], dtype=np.float32)
        in_maps.append({
            "xt": np.ascontiguousarray(x[b].T),
            "wqkv": wqkv, "bqk": bqk, "vb": vb, "wout": wout,
        })

    res = run_bass_kernel_spmd(nc, in_maps, list(range(N_CORES)), trace=TRACE)
    parts = [res.results[c]["out"] for c in range(N_CORES)]
    out = np.empty((B, S, D), dtype=np.float32)
    for b in range(B):
        out[b] = parts[2 * b] + parts[2 * b + 1] + out_b
    if TRACE:
        kernel.last_results = res
    return out



# revision 32
# speedup vs baseline: 1.1646x; 1.1646x over previous
"""Causal self-attention (B=4, S=2048, D=768, H=12) on 8 trn2 NeuronCores.

Sharding (Megatron-style): DP over the 4 batches x TP=2 over heads.
Core c handles batch c//2 with heads (c%2)*6 .. +6: qkv_proj column-parallel,
out_proj row-parallel; the TP pair's partial outputs are summed on the host.

Per-core kernel, all matmul operands bf16 (PSUM accumulation fp32):
  - qkT = (x @ Wqk)^T in [feat(part), s] layout, computed in 512-col quanta;
    bias applied on the PSUM->SBUF copy by DVE (tensor_scalar) so the ACT
    engine is reserved for exp. V in natural [s, feat] layout with a ones
    column per head (V') so PV also yields the softmax denominator.
  - attention per (head, 512-q-chunk): S^T tile = K_tile @ Q_chunk
    (contraction 64), exp on ACT batched 2 k-tiles per ACTIVATE -> pt bf16;
    causal via narrowed matmuls + per-diagonal-block mask mult (GPSIMD);
    PV as O[q(part),65] += pt_slice^T(stationary) @ V'(moving, N=65).
    PV flush is deferred TWO k-groups so exp results are always ready when
    the PE reaches them (no exp-latency stall), keeping ACT the pacer.
  - the engine-work gap between ACT (exp) and PE inside attention is filled
    from a queue of ~1us PE work quanta (remaining qkT columns, V tiles,
    O-transposes, out-proj tiles), paced by a static work-debt counter.
  - normalize per (h,qc): per-partition reciprocal of the ones column +
    tensor_scalar -> O_norm [q, feat] bf16; PE-transpose back to oT
    [feat(part), q]; out_partial = O @ Wout_slice, stored bf16 (host sums
    the TP pair in fp32).
"""
import numpy as np
import ml_dtypes
import concourse.bass as bass
import concourse.mybir as mybir
import concourse.tile as tile
from concourse import bacc
from concourse.bass_utils import run_bass_kernel_spmd
from concourse.masks import make_identity

B, S, D = 4, 2048, 768
H, HD = 12, 64
N_CORES = 8
HPC = H // 2          # heads per core = 6
FQK = HPC * HD        # 384 features per core for each of q,k,v
F32 = mybir.dt.float32
BF16 = mybir.dt.bfloat16

N_ST = S // 128       # 16 s tiles
N_QC = S // 512       # 4 q chunks
N_DT = D // 128       # 6 d_model tiles
WARMUP_MM = 20
PE_CY = 0.4167        # ns/row, warm

TRACE = False
_CACHE = {}
PHASE_MARKS = []      # (phase_name, first_inst_id)


def _mark(nc, name):
    PHASE_MARKS.append((name, nc.next_id()))


def _emit(nc):
    xt_d = nc.dram_tensor("xt", [D, S], BF16, kind="ExternalInput").ap()
    wqk_d = nc.dram_tensor("wqk", [D, 6 * 128], BF16, kind="ExternalInput").ap()
    wv_d = nc.dram_tensor("wv", [D, FQK], BF16, kind="ExternalInput").ap()
    bqk_d = nc.dram_tensor("bqk", [128, 6], F32, kind="ExternalInput").ap()
    vb_d = nc.dram_tensor("vb", [128, FQK], F32, kind="ExternalInput").ap()
    wout_d = nc.dram_tensor("wout", [FQK, D], BF16, kind="ExternalInput").ap()
    out_d = nc.dram_tensor("out", [S, D], BF16, kind="ExternalOutput").ap()

    with tile.TileContext(nc) as tc:
        with tc.tile_pool(name="const", bufs=1) as pc, \
             tc.tile_pool(name="xTp", bufs=1) as pxt, \
             tc.tile_pool(name="qkTp", bufs=1) as pqk, \
             tc.tile_pool(name="vnp", bufs=1) as pvn, \
             tc.tile_pool(name="wstr", bufs=3) as pw, \
             tc.tile_pool(name="ptp", bufs=4) as ppt, \
             tc.tile_pool(name="onp", bufs=2) as pon, \
             tc.tile_pool(name="rp", bufs=2) as prp, \
             tc.tile_pool(name="oTp", bufs=1) as pot, \
             tc.tile_pool(name="outp", bufs=2) as pout, \
             tc.tile_pool(name="ps", bufs=2, space="PSUM") as pp, \
             tc.tile_pool(name="psv", bufs=2, space="PSUM") as ppv, \
             tc.tile_pool(name="psd", bufs=2, space="PSUM") as ppd:

            # ---- constants ----
            warm = pc.tile([128, 128], BF16)
            nc.vector.memset(warm[:], 0.25)
            identity = pc.tile([128, 128], BF16)
            make_identity(nc, identity)
            mask = pc.tile([128, 128], BF16)
            nc.gpsimd.memset(mask[:], 1.0)
            nc.gpsimd.affine_select(
                out=mask[:], in_=mask[:], compare_op=mybir.AluOpType.is_ge,
                fill=0.0, base=0, channel_multiplier=-1, pattern=[[1, 128]])

            bqk_sb = pc.tile([128, 6], F32)
            vb_sb = pc.tile([128, FQK], F32)
            wv_sb = pc.tile([128, N_DT, FQK], BF16)
            wout_sb = pc.tile([128, FQK // 128, D], BF16)

            xT = pxt.tile([128, N_DT, S], BF16)
            qkT = pqk.tile([128, 6, S], BF16)       # ft: q0,k0,q1,k1,q2,k2
            vn = pvn.tile([128, N_ST, HPC, HD + 1], BF16)
            oT = pot.tile([128, FQK // 128, S], BF16)

            _mark(nc, "A:load")
            # PE warmup: ramp the pstate + cover initial DMA latency
            for i in range(WARMUP_MM):
                ps_wm = ppd.tile([128, 512], F32, tag="d", name="ps_wm")
                nc.tensor.matmul(ps_wm[:, 0:128], warm[:], warm[:],
                                 start=True, stop=True)

            w_ts = {}

            def load_wpair(p):
                w_t = pw.tile([128, N_DT, 256], BF16, tag="w",
                              name=f"w_t{p}")
                nc.sync.dma_start(
                    w_t[:],
                    wqk_d[:, p * 256:(p + 1) * 256].rearrange(
                        "(t p) f -> p t f", p=128))
                w_ts[p] = w_t

            nc.sync.dma_start(bqk_sb[:], bqk_d[:])
            nc.sync.dma_start(
                xT[:, 0:3, 0:512],
                xt_d[0:384, 0:512].rearrange("(t p) s -> p t s", p=128))
            load_wpair(0)
            nc.sync.dma_start(
                xT[:, 3:6, 0:512],
                xt_d[384:768, 0:512].rearrange("(t p) s -> p t s", p=128))
            nc.sync.dma_start(
                wv_sb[:], wv_d.rearrange("(t p) f -> p t f", p=128))
            nc.sync.dma_start(vb_sb[:], vb_d[:])
            nc.sync.dma_start(
                xT[:, :, 512:1024],
                xt_d[:, 512:1024].rearrange("(t p) s -> p t s", p=128))
            load_wpair(1)
            nc.sync.dma_start(
                xT[:, :, 1024:1536],
                xt_d[:, 1024:1536].rearrange("(t p) s -> p t s", p=128))
            load_wpair(2)
            nc.sync.dma_start(
                xT[:, :, 1536:2048],
                xt_d[:, 1536:2048].rearrange("(t p) s -> p t s", p=128))
            nc.sync.dma_start(
                wout_sb[:], wout_d.rearrange("(t p) o -> p t o", p=128))

            nc.vector.memset(vn[:, :, :, HD:HD + 1], 1.0)
            vb_h = vb_sb.rearrange("p (h d) -> p h d", d=HD)

            # ---- work quanta ----
            def emit_v(st):
                ps_v = ppd.tile([128, 512], F32, tag="d", name="ps_v")
                for dc in range(N_DT):
                    nc.tensor.matmul(
                        ps_v[:, 0:FQK],
                        xT[:, dc, st * 128:(st + 1) * 128],
                        wv_sb[:, dc, :],
                        start=(dc == 0), stop=(dc == N_DT - 1))
                nc.vector.tensor_tensor(
                    vn[:, st, :, 0:HD],
                    ps_v[:, 0:FQK].rearrange("p (h d) -> p h d", d=HD),
                    vb_h, mybir.AluOpType.add)

            def emit_qk(ft, ch, pool_tag="d"):
                # qkT[:, ft, ch*512:(ch+1)*512] = (x @ w_ft)^T + b_ft
                p, fip = ft // 2, ft % 2
                w_t = w_ts[p]
                pool = ppd if pool_tag == "d" else pp
                ps_qk = pool.tile([128, 512], F32, tag=pool_tag,
                                  name="ps_qk")
                for dc in range(N_DT):
                    nc.tensor.matmul(
                        ps_qk[:],
                        w_t[:, dc, fip * 128:(fip + 1) * 128],
                        xT[:, dc, ch * 512:(ch + 1) * 512],
                        start=(dc == 0), stop=(dc == N_DT - 1))
                nc.vector.tensor_scalar(
                    qkT[:, ft, ch * 512:(ch + 1) * 512], ps_qk[:],
                    bqk_sb[:, ft:ft + 1], None, mybir.AluOpType.add)

            def transpose_o(qc, qt):
                onorm = onorms[qc]
                for k in range(FQK // 128):
                    tp = ppd.tile([128, 128], BF16, tag="d", name="tp")
                    nc.tensor.transpose(
                        tp[:], onorm[:, qt, k * 128:(k + 1) * 128],
                        identity[:])
                    nc.vector.tensor_copy(
                        oT[:, k, (qc * 4 + qt) * 128:(qc * 4 + qt + 1) * 128],
                        tp[:])

            def emit_proj(st):
                o_sb = pout.tile([128, D], BF16, tag="o_sb", name="o_sb")
                for oc in range(2):
                    ps_d = ppd.tile([128, 384], F32, tag="d", name="ps_d")
                    for ht in range(FQK // 128):
                        nc.tensor.matmul(
                            ps_d[:],
                            oT[:, ht, st * 128:(st + 1) * 128],
                            wout_sb[:, ht, oc * 384:(oc + 1) * 384],
                            start=(ht == 0),
                            stop=(ht == FQK // 128 - 1))
                    nc.vector.tensor_copy(
                        o_sb[:, oc * 384:(oc + 1) * 384], ps_d[:])
                nc.sync.dma_start(
                    out_d[st * 128:(st + 1) * 128, :], o_sb[:])

            # ---- filler queue: (kind, args, pe_ns) popped by work debt ----
            filler = []

            def filler_run(item):
                kind, args = item[0], item[1]
                if kind == "v":
                    emit_v(args)
                elif kind == "qk":
                    emit_qk(*args)
                elif kind == "tproj":
                    qc, qt = args
                    transpose_o(qc, qt)
                    emit_proj(qc * 4 + qt)

            QK_NS = 6 * 512 * PE_CY
            V_NS = 6 * 384 * PE_CY
            TPROJ_NS = (3 * 128 + 6 * 384) * PE_CY

            vdone = {st: False for st in range(16)}
            qkdone = set()

            def note_done(item):
                if item[0] == "v":
                    vdone[item[1]] = True
                elif item[0] == "qk":
                    qkdone.add(item[1])

            state = {"debt": 0.0}

            def pump(ns):
                state["debt"] += ns
                while filler and state["debt"] >= 0.6 * filler[0][2]:
                    item = filler.pop(0)
                    state["debt"] -= item[2]
                    filler_run(item)
                    note_done(item)

            def force(need_v_st, need_qk):
                # selectively pull prerequisite items out of the filler
                need = [("v", st) for st in need_v_st if not vdone[st]]
                need += [("qk", q) for q in need_qk if q not in qkdone]
                for key in need:
                    for i, item in enumerate(filler):
                        if (item[0], item[1]) == key:
                            filler.pop(i)
                            filler_run(item)
                            note_done(item)
                            break

            # ---- attention ----
            pend = []   # deferred PV-flush closures, depth 2
            onorms = {}

            def attn_job(h, qc, inject=(), tail_mode=False):
                po = (h % 2) * 64
                qft = 2 * (h // 2)
                kft = qft + 1
                n_kt = 4 * (qc + 1)
                inject = list(inject)
                pv = ppv.tile([128, 4, HD + 1], F32, tag="v", name="pv")

                def mk_flush(prev_grp, last):
                    def flush():
                        ktg, offs, pt = prev_grp
                        for j in range(2):
                            kt = ktg + j
                            q_off = offs[j]
                            for qt in range(4):
                                q_lo = qt * 128
                                if q_lo < q_off:
                                    continue
                                # start only on the job's first matmul: on
                                # HW, start=True clears has_written for the
                                # WHOLE bank, which would wipe accumulation
                                # continuity of the other q-tiles' groups
                                nc.tensor.matmul(
                                    pv[:, qt, :],
                                    pt[:, j * 512 + q_lo:j * 512 + q_lo + 128],
                                    vn[:, kt, h, :],
                                    start=(kt == 0 and qt == 0),
                                    stop=(kt == 4 * qc + qt))
                        if tail_mode:
                            # drain each q-tile the moment its diagonal
                            # k-tile lands: normalize + transpose + out-proj
                            for j in range(2):
                                qt = ktg + j - 4 * qc
                                if 0 <= qt < 4:
                                    normalize_qt(h, qc, pv, qt)
                                    transpose_o(qc, qt)
                                    emit_proj(qc * 4 + qt)
                        elif last:
                            job_done(h, qc, pv)
                    return flush

                for ktg in range(0, n_kt, 2):
                    ps_s = pp.tile([128, 1024], F32, tag="s", name="ps_s")
                    offs = []
                    for j in range(2):
                        kt = ktg + j
                        q_off = max(0, kt * 128 - qc * 512)
                        offs.append(q_off)
                        nc.tensor.matmul(
                            ps_s[:, j * 512 + q_off:(j + 1) * 512],
                            qkT[po:po + 64, kft, kt * 128:(kt + 1) * 128],
                            qkT[po:po + 64, qft,
                                qc * 512 + q_off:(qc + 1) * 512],
                            start=True, stop=True)
                    pt = ppt.tile([128, 1024], BF16, tag="pt", name="pt")
                    nc.scalar.activation(
                        pt[:, offs[0]:], ps_s[:, offs[0]:],
                        mybir.ActivationFunctionType.Exp)
                    for j in range(2):
                        kt = ktg + j
                        if kt >= 4 * qc:
                            qt_d = kt - 4 * qc
                            sl = slice(j * 512 + qt_d * 128,
                                       j * 512 + qt_d * 128 + 128)
                            nc.gpsimd.tensor_tensor(
                                pt[:, sl], pt[:, sl], mask[:],
                                mybir.AluOpType.mult)
                    for _ in range(2):
                        if inject:
                            item = inject.pop(0)
                            filler_run(item)
                            note_done(item)
                            state["debt"] -= item[2]
                    # pump BEFORE the deferred-PV pop: filler work is always
                    # ready, while PV flushes may still wait on exp — behind
                    # them in the in-order queue, ready work would stall
                    grp_rows = (512 - offs[0]) + (512 - offs[1])
                    pv_rows = sum(65 for j in range(2) for qt in range(4)
                                  if qt * 128 >= offs[j])
                    act_ns = (1024 - offs[0]) * 0.833 + 217
                    pump(act_ns - (grp_rows + pv_rows) * PE_CY)
                    while len(pend) >= (1 if tail_mode else 2):
                        pend.pop(0)()
                    pend.append(mk_flush((ktg, offs, pt),
                                         ktg == n_kt - 2))
                for item in inject:
                    filler_run(item)
                    note_done(item)
                    state["debt"] -= item[2]

            def normalize_qt(h, qc, pv, qt):
                onorm = onorms[qc]
                r1 = prp.tile([128, 1], F32, tag="r1", name="r1")
                nc.vector.reciprocal(r1[:], pv[:, qt, HD:HD + 1])
                nc.vector.tensor_scalar(
                    onorm[:, qt, h * HD:(h + 1) * HD],
                    pv[:, qt, 0:HD], r1[:], None, mybir.AluOpType.mult)

            def job_done(h, qc, pv):
                # job h's accumulation is complete: normalize into onorm
                if qc not in onorms:
                    onorms[qc] = pon.tile([128, 4, FQK], BF16, tag="on",
                                          name=f"onorm{qc}")
                onorm = onorms[qc]
                r = prp.tile([128, 4], F32, tag="r", name="r")
                nc.vector.reciprocal(r[:], pv[:, :, HD])
                for qt in range(4):
                    nc.vector.tensor_scalar(
                        onorm[:, qt, h * HD:(h + 1) * HD],
                        pv[:, qt, 0:HD], r[:, qt:qt + 1], None,
                        mybir.AluOpType.mult)
                if h == HPC - 1:
                    for qt in range(4):
                        filler.append(("tproj", (qc, qt), TPROJ_NS))

            _mark(nc, "B:start")
            emit_qk(0, 0, pool_tag="s")
            emit_qk(1, 0, pool_tag="s")

            # all remaining qkT columns / V tiles as per-job injected quanta,
            # each placed with slack before its deadline (see flush timing)
            QK = lambda ft, ch: ("qk", (ft, ch), QK_NS)
            VQ = lambda st: ("v", st, V_NS)
            inject_map = {
                (0, 0): [VQ(0), VQ(1), VQ(2), VQ(3)],
                (0, 1): [QK(2, 0), QK(3, 0)],
                (0, 2): [QK(4, 0)],
                (0, 3): [QK(5, 0)],
                (0, 4): [QK(0, 1), QK(1, 1)],
                (0, 5): [VQ(4), VQ(5), VQ(6), VQ(7)],
                (1, 0): [QK(2, 1), QK(3, 1)],
                (1, 1): [QK(4, 1), QK(5, 1)],
                (1, 2): [QK(0, 2), QK(1, 2)],
                (1, 3): [VQ(8), VQ(9), VQ(10), VQ(11)],
                (1, 4): [QK(2, 2), QK(3, 2)],
                (1, 5): [QK(4, 2), QK(5, 2)],
                (2, 0): [QK(0, 3), QK(1, 3)],
                (2, 1): [VQ(12), VQ(13), VQ(14), VQ(15)],
                (2, 2): [QK(2, 3), QK(3, 3)],
                (2, 3): [QK(4, 3), QK(5, 3)],
            }
            for qc in range(N_QC):
                _mark(nc, f"C:attn qc={qc}")
                for h in range(HPC):
                    need_qk = [(2 * (h // 2), ch) for ch in range(qc + 1)]
                    need_qk += [(2 * (h // 2) + 1, ch) for ch in range(qc + 1)]
                    need_qk = [q for q in need_qk if q[1] > 0]
                    # vn tiles are first read by the previous job's deferred
                    # PV flushes, so the requirement lags one job
                    v_hi = 4 * qc + 4 if h >= 1 else 4 * qc
                    force([st for st in range(4, v_hi)], need_qk)
                    attn_job(h, qc, inject=inject_map.get((qc, h), ()),
                             tail_mode=(qc == N_QC - 1 and h == HPC - 1))

            _mark(nc, "D:tail")
            while pend:
                pend.pop(0)()
            while filler:
                item = filler.pop(0)
                filler_run(item)


def _build():
    if "nc" not in _CACHE:
        nc = bacc.Bacc("TRN2", target_bir_lowering=False, debug=False,
                       num_devices=N_CORES)
        _emit(nc)
        nc.compile()
        _CACHE["nc"] = nc
    return _CACHE["nc"]


def _bf16(a):
    return np.ascontiguousarray(a.astype(ml_dtypes.bfloat16))


def kernel(x, qkv_w, qkv_b, out_w, out_b):
    x = np.asarray(x, dtype=np.float32)
    qkv_w = np.asarray(qkv_w, dtype=np.float32)
    qkv_b = np.asarray(qkv_b, dtype=np.float32)
    out_w = np.asarray(out_w, dtype=np.float32)
    out_b = np.asarray(out_b, dtype=np.float32)

    nc = _build()
    scale = HD ** -0.5
    in_maps = []
    for c in range(N_CORES):
        b, half = c // 2, c % 2
        wq = qkv_w[:, half * FQK:(half + 1) * FQK] * scale
        wk = qkv_w[:, D + half * FQK:D + (half + 1) * FQK]
        wv = qkv_w[:, 2 * D + half * FQK:2 * D + (half + 1) * FQK]
        bq = qkv_b[half * FQK:(half + 1) * FQK] * scale
        bk = qkv_b[D + half * FQK:D + (half + 1) * FQK]
        bv = qkv_b[2 * D + half * FQK:2 * D + (half + 1) * FQK]
        # pair-interleaved qk weights: [q0,k0,q1,k1,q2,k2] blocks of 128
        wqk = np.empty((D, 6 * 128), dtype=np.float32)
        bqk = np.empty((6, 128), dtype=np.float32)
        for p in range(3):
            wqk[:, (2 * p) * 128:(2 * p + 1) * 128] = \
                wq[:, p * 128:(p + 1) * 128]
            wqk[:, (2 * p + 1) * 128:(2 * p + 2) * 128] = \
                wk[:, p * 128:(p + 1) * 128]
            bqk[2 * p] = bq[p * 128:(p + 1) * 128]
            bqk[2 * p + 1] = bk[p * 128:(p + 1) * 128]
        in_maps.append({
            "xt": _bf16(x[b].T),
            "wqk": _bf16(wqk),
            "wv": _bf16(wv),
            "bqk": np.ascontiguousarray(bqk.T, dtype=np.float32),
            "vb": np.ascontiguousarray(
                np.broadcast_to(bv, (128, FQK)), dtype=np.float32),
            "wout": _bf16(out_w[half * FQK:(half + 1) * FQK, :]),
        })

    res = run_bass_kernel_spmd(nc, in_maps, list(range(N_CORES)), trace=TRACE)
    parts = [np.asarray(res.results[c]["out"], dtype=np.float32)
             for c in range(N_CORES)]
    out = np.empty((B, S, D), dtype=np.float32)
    for b in range(B):
        out[b] = parts[2 * b] + parts[2 * b + 1] + out_b
    if TRACE:
        kernel.last_results = res
    return out


# revision 38
# speedup vs baseline: 1.2265x; 1.0532x over previous
"""Causal self-attention (B=4, S=2048, D=768, H=12) on 8 trn2 NeuronCores.

Sharding (Megatron-style): DP over the 4 batches x TP=2 over heads.
Core c handles batch c//2 with heads (c%2)*6 .. +6: qkv_proj column-parallel,
out_proj row-parallel; the TP pair's partial outputs are summed on the host.

Per-core kernel, all matmul operands bf16 (PSUM accumulation fp32):
  - qkT = (x @ Wqk)^T in [feat(part), s] layout, computed in 512-col quanta;
    bias applied on the PSUM->SBUF copy by DVE (tensor_scalar) so the ACT
    engine is reserved for exp. V in natural [s, feat] layout with a ones
    column per head (V') so PV also yields the softmax denominator.
  - attention per (head, 512-q-chunk): S^T tile = K_tile @ Q_chunk
    (contraction 64), exp on ACT batched 2 k-tiles per ACTIVATE -> pt bf16;
    causal via narrowed matmuls + per-diagonal-block mask mult (GPSIMD);
    PV as O[q(part),65] += pt_slice^T(stationary) @ V'(moving, N=65).
    PV flush is deferred TWO k-groups so exp results are always ready when
    the PE reaches them (no exp-latency stall), keeping ACT the pacer.
  - the engine-work gap between ACT (exp) and PE inside attention is filled
    from a queue of ~1us PE work quanta (remaining qkT columns, V tiles,
    O-transposes, out-proj tiles), paced by a static work-debt counter.
  - normalize per (h,qc): per-partition reciprocal of the ones column +
    tensor_scalar -> O_norm [q, feat] bf16; PE-transpose back to oT
    [feat(part), q]; out_partial = O @ Wout_slice, stored bf16 (host sums
    the TP pair in fp32).
"""
import numpy as np
import ml_dtypes
import concourse.bass as bass
import concourse.mybir as mybir
import concourse.tile as tile
from concourse import bacc
from concourse.bass_utils import run_bass_kernel_spmd
from concourse.masks import make_identity

B, S, D = 4, 2048, 768
H, HD = 12, 64
N_CORES = 8
HPC = H // 2          # heads per core = 6
FQK = HPC * HD        # 384 features per core for each of q,k,v
F32 = mybir.dt.float32
BF16 = mybir.dt.bfloat16

N_ST = S // 128       # 16 s tiles
N_QC = S // 512       # 4 q chunks
N_DT = D // 128       # 6 d_model tiles
WARMUP_MM = 20
PE_CY = 0.4167        # ns/row, warm

TRACE = False
_CACHE = {}
PHASE_MARKS = []      # (phase_name, first_inst_id)


def _mark(nc, name):
    PHASE_MARKS.append((name, nc.next_id()))


def _emit(nc):
    xt_d = nc.dram_tensor("xt", [D, S], BF16, kind="ExternalInput").ap()
    wqk_d = nc.dram_tensor("wqk", [D, 6 * 128], BF16, kind="ExternalInput").ap()
    wv_d = nc.dram_tensor("wv", [D, FQK], BF16, kind="ExternalInput").ap()
    bqk_d = nc.dram_tensor("bqk", [128, 6], F32, kind="ExternalInput").ap()
    vb_d = nc.dram_tensor("vb", [128, FQK], F32, kind="ExternalInput").ap()
    wout_d = nc.dram_tensor("wout", [FQK, D], BF16, kind="ExternalInput").ap()
    out_d = nc.dram_tensor("out", [S, D], BF16, kind="ExternalOutput").ap()

    with tile.TileContext(nc) as tc:
        with tc.tile_pool(name="const", bufs=1) as pc, \
             tc.tile_pool(name="xTp", bufs=1) as pxt, \
             tc.tile_pool(name="qkTp", bufs=1) as pqk, \
             tc.tile_pool(name="vnp", bufs=1) as pvn, \
             tc.tile_pool(name="wstr", bufs=3) as pw, \
             tc.tile_pool(name="ptp", bufs=6) as ppt, \
             tc.tile_pool(name="onp", bufs=2) as pon, \
             tc.tile_pool(name="rp", bufs=2) as prp, \
             tc.tile_pool(name="oTp", bufs=1) as pot, \
             tc.tile_pool(name="outp", bufs=2) as pout, \
             tc.tile_pool(name="ps", bufs=2, space="PSUM") as pp, \
             tc.tile_pool(name="psv", bufs=2, space="PSUM") as ppv, \
             tc.tile_pool(name="psd", bufs=2, space="PSUM") as ppd:

            # ---- constants ----
            warm = pc.tile([128, 128], BF16)
            nc.vector.memset(warm[:], 0.25)
            identity = pc.tile([128, 128], BF16)
            make_identity(nc, identity)
            mask = pc.tile([128, 128], BF16)
            nc.gpsimd.memset(mask[:], 1.0)
            nc.gpsimd.affine_select(
                out=mask[:], in_=mask[:], compare_op=mybir.AluOpType.is_ge,
                fill=0.0, base=0, channel_multiplier=-1, pattern=[[1, 128]])

            bqk_sb = pc.tile([128, 6], F32)
            vb_sb = pc.tile([128, FQK], F32)
            wv_sb = pc.tile([128, N_DT, FQK], BF16)
            wout_sb = pc.tile([128, FQK // 128, D], BF16)

            xT = pxt.tile([128, N_DT, S], BF16)
            qkT = pqk.tile([128, 6, S], BF16)       # ft: q0,k0,q1,k1,q2,k2
            vn = pvn.tile([128, N_ST, HPC, HD + 1], BF16)
            oT = pot.tile([128, FQK // 128, S], BF16)

            _mark(nc, "A:load")
            # PE warmup: ramp the pstate + cover initial DMA latency
            for i in range(WARMUP_MM):
                ps_wm = ppd.tile([128, 512], F32, tag="d", name="ps_wm")
                nc.tensor.matmul(ps_wm[:, 0:128], warm[:], warm[:],
                                 start=True, stop=True)

            w_ts = {}

            def load_wpair(p):
                w_t = pw.tile([128, N_DT, 256], BF16, tag="w",
                              name=f"w_t{p}")
                nc.sync.dma_start(
                    w_t[:],
                    wqk_d[:, p * 256:(p + 1) * 256].rearrange(
                        "(t p) f -> p t f", p=128))
                w_ts[p] = w_t

            nc.sync.dma_start(bqk_sb[:], bqk_d[:])
            nc.sync.dma_start(
                xT[:, 0:3, 0:512],
                xt_d[0:384, 0:512].rearrange("(t p) s -> p t s", p=128))
            load_wpair(0)
            nc.sync.dma_start(
                xT[:, 3:6, 0:512],
                xt_d[384:768, 0:512].rearrange("(t p) s -> p t s", p=128))
            nc.sync.dma_start(
                wv_sb[:], wv_d.rearrange("(t p) f -> p t f", p=128))
            nc.sync.dma_start(vb_sb[:], vb_d[:])
            nc.sync.dma_start(
                xT[:, :, 512:1024],
                xt_d[:, 512:1024].rearrange("(t p) s -> p t s", p=128))
            load_wpair(1)
            nc.sync.dma_start(
                xT[:, :, 1024:1536],
                xt_d[:, 1024:1536].rearrange("(t p) s -> p t s", p=128))
            load_wpair(2)
            nc.sync.dma_start(
                xT[:, :, 1536:2048],
                xt_d[:, 1536:2048].rearrange("(t p) s -> p t s", p=128))
            nc.sync.dma_start(
                wout_sb[:], wout_d.rearrange("(t p) o -> p t o", p=128))

            nc.vector.memset(vn[:, :, :, HD:HD + 1], 1.0)
            vb_h = vb_sb.rearrange("p (h d) -> p h d", d=HD)

            # ---- work quanta ----
            def emit_v(st):
                ps_v = ppd.tile([128, 512], F32, tag="d", name="ps_v")
                for dc in range(N_DT):
                    nc.tensor.matmul(
                        ps_v[:, 0:FQK],
                        xT[:, dc, st * 128:(st + 1) * 128],
                        wv_sb[:, dc, :],
                        start=(dc == 0), stop=(dc == N_DT - 1))
                nc.vector.tensor_tensor(
                    vn[:, st, :, 0:HD],
                    ps_v[:, 0:FQK].rearrange("p (h d) -> p h d", d=HD),
                    vb_h, mybir.AluOpType.add)

            def emit_qk(ft, ch, pool_tag="d"):
                # qkT[:, ft, ch*512:(ch+1)*512] = (x @ w_ft)^T + b_ft
                p, fip = ft // 2, ft % 2
                w_t = w_ts[p]
                pool = ppd if pool_tag == "d" else pp
                ps_qk = pool.tile([128, 512], F32, tag=pool_tag,
                                  name="ps_qk")
                for dc in range(N_DT):
                    nc.tensor.matmul(
                        ps_qk[:],
                        w_t[:, dc, fip * 128:(fip + 1) * 128],
                        xT[:, dc, ch * 512:(ch + 1) * 512],
                        start=(dc == 0), stop=(dc == N_DT - 1))
                nc.vector.tensor_scalar(
                    qkT[:, ft, ch * 512:(ch + 1) * 512], ps_qk[:],
                    bqk_sb[:, ft:ft + 1], None, mybir.AluOpType.add)

            def transpose_o(qc, qt):
                onorm = onorms[qc]
                for k in range(FQK // 128):
                    tp = ppd.tile([128, 128], BF16, tag="d", name="tp")
                    nc.tensor.transpose(
                        tp[:], onorm[:, qt, k * 128:(k + 1) * 128],
                        identity[:])
                    nc.vector.tensor_copy(
                        oT[:, k, (qc * 4 + qt) * 128:(qc * 4 + qt + 1) * 128],
                        tp[:])

            def emit_proj(st):
                o_sb = pout.tile([128, D], BF16, tag="o_sb", name="o_sb")
                for oc in range(2):
                    ps_d = ppd.tile([128, 384], F32, tag="d", name="ps_d")
                    for ht in range(FQK // 128):
                        nc.tensor.matmul(
                            ps_d[:],
                            oT[:, ht, st * 128:(st + 1) * 128],
                            wout_sb[:, ht, oc * 384:(oc + 1) * 384],
                            start=(ht == 0),
                            stop=(ht == FQK // 128 - 1))
                    nc.vector.tensor_copy(
                        o_sb[:, oc * 384:(oc + 1) * 384], ps_d[:])
                    nc.sync.dma_start(
                        out_d[st * 128:(st + 1) * 128,
                              oc * 384:(oc + 1) * 384],
                        o_sb[:, oc * 384:(oc + 1) * 384])

            # ---- filler queue: (kind, args, pe_ns) popped by work debt ----
            filler = []

            def filler_run(item):
                kind, args = item[0], item[1]
                if kind == "v":
                    emit_v(args)
                elif kind == "qk":
                    emit_qk(*args)
                elif kind == "tproj":
                    qc, qt = args
                    transpose_o(qc, qt)
                    emit_proj(qc * 4 + qt)

            QK_NS = 6 * 512 * PE_CY
            V_NS = 6 * 384 * PE_CY
            TPROJ_NS = (3 * 128 + 6 * 384) * PE_CY

            vdone = {st: False for st in range(16)}
            qkdone = set()

            def note_done(item):
                if item[0] == "v":
                    vdone[item[1]] = True
                elif item[0] == "qk":
                    qkdone.add(item[1])

            state = {"debt": 0.0}

            def pump(ns):
                # clamp: deep-negative debt would starve the queue for whole
                # q-chunks; large-positive would dump bursts that starve ACT
                state["debt"] = min(max(state["debt"] + ns, -2000.0), 2400.0)
                while filler and state["debt"] >= 0.6 * filler[0][2]:
                    item = filler.pop(0)
                    state["debt"] -= item[2]
                    filler_run(item)
                    note_done(item)

            def force(need_v_st, need_qk):
                # selectively pull prerequisite items out of the filler
                need = [("v", st) for st in need_v_st if not vdone[st]]
                need += [("qk", q) for q in need_qk if q not in qkdone]
                for key in need:
                    for i, item in enumerate(filler):
                        if (item[0], item[1]) == key:
                            filler.pop(i)
                            filler_run(item)
                            note_done(item)
                            break

            # ---- attention ----
            pend = []   # deferred PV-flush closures, depth 2
            onorms = {}

            def attn_job(h, qc, inject=(), tail_mode=False):
                po = (h % 2) * 64
                qft = 2 * (h // 2)
                kft = qft + 1
                n_kt = 4 * (qc + 1)
                inject = list(inject)
                pv = ppv.tile([128, 4, HD + 1], F32, tag="v", name="pv")

                def mk_flush(prev_grp, last):
                    def flush():
                        ktg, offs, pt = prev_grp
                        for j in range(2):
                            kt = ktg + j
                            q_off = offs[j]
                            for qt in range(4):
                                q_lo = qt * 128
                                if q_lo < q_off:
                                    continue
                                # start only on the job's first matmul: on
                                # HW, start=True clears has_written for the
                                # WHOLE bank, which would wipe accumulation
                                # continuity of the other q-tiles' groups
                                nc.tensor.matmul(
                                    pv[:, qt, :],
                                    pt[:, j * 512 + q_lo:j * 512 + q_lo + 128],
                                    vn[:, kt, h, :],
                                    start=(kt == 0 and qt == 0),
                                    stop=(kt == 4 * qc + qt))
                        if tail_mode:
                            # drain each q-tile the moment its diagonal
                            # k-tile lands: normalize + transpose + out-proj,
                            # interleaved across the group's two q-tiles so
                            # PE transposes overlap the other tile's copies
                            qts = [ktg + j - 4 * qc for j in range(2)
                                   if 0 <= ktg + j - 4 * qc < 4]
                            for qt in qts:
                                normalize_qt(h, qc, pv, qt)
                            for qt in qts:
                                transpose_o(qc, qt)
                            for qt in qts:
                                emit_proj(qc * 4 + qt)
                        elif last:
                            job_done(h, qc, pv)
                    return flush

                for ktg in range(0, n_kt, 2):
                    ps_s = pp.tile([128, 1024], F32, tag="s", name="ps_s")
                    offs = []
                    for j in range(2):
                        kt = ktg + j
                        q_off = max(0, kt * 128 - qc * 512)
                        offs.append(q_off)
                        nc.tensor.matmul(
                            ps_s[:, j * 512 + q_off:(j + 1) * 512],
                            qkT[po:po + 64, kft, kt * 128:(kt + 1) * 128],
                            qkT[po:po + 64, qft,
                                qc * 512 + q_off:(qc + 1) * 512],
                            start=True, stop=True)
                    pt = ppt.tile([128, 1024], BF16, tag="pt", name="pt")
                    nc.scalar.activation(
                        pt[:, offs[0]:], ps_s[:, offs[0]:],
                        mybir.ActivationFunctionType.Exp)
                    for j in range(2):
                        kt = ktg + j
                        if kt >= 4 * qc:
                            qt_d = kt - 4 * qc
                            sl = slice(j * 512 + qt_d * 128,
                                       j * 512 + qt_d * 128 + 128)
                            nc.gpsimd.tensor_tensor(
                                pt[:, sl], pt[:, sl], mask[:],
                                mybir.AluOpType.mult)
                    for _ in range(2):
                        if inject:
                            item = inject.pop(0)
                            filler_run(item)
                            note_done(item)
                            state["debt"] -= item[2]
                    # pump BEFORE the deferred-PV pop: filler work is always
                    # ready, while PV flushes may still wait on exp — behind
                    # them in the in-order queue, ready work would stall
                    grp_rows = (512 - offs[0]) + (512 - offs[1])
                    pv_rows = sum(65 for j in range(2) for qt in range(4)
                                  if qt * 128 >= offs[j])
                    act_ns = (1024 - offs[0]) * 0.833 + 217
                    pump(act_ns - (grp_rows + pv_rows) * PE_CY)
                    while len(pend) >= (1 if tail_mode else 2):
                        pend.pop(0)()
                    pend.append(mk_flush((ktg, offs, pt),
                                         ktg == n_kt - 2))
                for item in inject:
                    filler_run(item)
                    note_done(item)
                    state["debt"] -= item[2]

            def normalize_qt(h, qc, pv, qt):
                onorm = onorms[qc]
                r1 = prp.tile([128, 1], F32, tag="r1", name="r1")
                nc.vector.reciprocal(r1[:], pv[:, qt, HD:HD + 1])
                nc.vector.tensor_scalar(
                    onorm[:, qt, h * HD:(h + 1) * HD],
                    pv[:, qt, 0:HD], r1[:], None, mybir.AluOpType.mult)

            def job_done(h, qc, pv):
                # job h's accumulation is complete: normalize into onorm
                if qc not in onorms:
                    onorms[qc] = pon.tile([128, 4, FQK], BF16, tag="on",
                                          name=f"onorm{qc}")
                onorm = onorms[qc]
                r = prp.tile([128, 4], F32, tag="r", name="r")
                nc.vector.reciprocal(r[:], pv[:, :, HD])
                for qt in range(4):
                    nc.vector.tensor_scalar(
                        onorm[:, qt, h * HD:(h + 1) * HD],
                        pv[:, qt, 0:HD], r[:, qt:qt + 1], None,
                        mybir.AluOpType.mult)
                if h == HPC - 1:
                    for qt in range(4):
                        filler.append(("tproj", (qc, qt), TPROJ_NS))

            _mark(nc, "B:start")
            emit_qk(0, 0, pool_tag="s")
            emit_qk(1, 0, pool_tag="s")

            # all remaining qkT columns / V tiles as per-job injected quanta,
            # each placed with slack before its deadline (see flush timing)
            QK = lambda ft, ch: ("qk", (ft, ch), QK_NS)
            VQ = lambda st: ("v", st, V_NS)
            inject_map = {
                (0, 0): [VQ(0), VQ(1), VQ(2), VQ(3)],
                (0, 1): [QK(2, 0), QK(3, 0)],
                (0, 2): [QK(4, 0)],
                (0, 3): [QK(5, 0)],
                (0, 4): [QK(0, 1), QK(1, 1)],
                (0, 5): [VQ(4), VQ(5), VQ(6), VQ(7)],
                (1, 0): [QK(2, 1), QK(3, 1)],
                (1, 1): [QK(4, 1), QK(5, 1)],
                (1, 2): [QK(0, 2), QK(1, 2)],
                (1, 3): [VQ(8), VQ(9), VQ(10), VQ(11)],
                (1, 4): [QK(2, 2), QK(3, 2)],
                (1, 5): [QK(4, 2), QK(5, 2)],
                (2, 0): [QK(0, 3), QK(1, 3)],
                # qc3 is ACT-bound with PE slack; push deadline-permitting
                # work there (V12-15 first read at job(1,3), qk ch3 for
                # heads 2+ read at jobs (2..5,3))
                (3, 0): [VQ(12), VQ(13), VQ(14), VQ(15),
                         QK(2, 3), QK(3, 3)],
                (3, 1): [QK(4, 3), QK(5, 3)],
            }
            for qc in range(N_QC):
                _mark(nc, f"C:attn qc={qc}")
                # onorm buffers rotate with depth 2: qc's normalize writes
                # reuse qc-2's buffer, so qc-2's transposes must be emitted
                for item in [f for f in filler
                             if f[0] == "tproj" and f[1][0] == qc - 2]:
                    filler.remove(item)
                    filler_run(item)
                for h in range(HPC):
                    need_qk = [(2 * (h // 2), ch) for ch in range(qc + 1)]
                    need_qk += [(2 * (h // 2) + 1, ch) for ch in range(qc + 1)]
                    need_qk = [q for q in need_qk if q[1] > 0]
                    # vn tiles are first read by the previous job's deferred
                    # PV flushes, so the requirement lags one job
                    v_hi = 4 * qc + 4 if h >= 1 else 4 * qc
                    force([st for st in range(4, v_hi)], need_qk)
                    attn_job(h, qc, inject=inject_map.get((qc, h), ()),
                             tail_mode=(qc == N_QC - 1 and h == HPC - 1))

            _mark(nc, "D:tail")
            while pend:
                pend.pop(0)()
            while filler:
                item = filler.pop(0)
                filler_run(item)


def _build():
    if "nc" not in _CACHE:
        nc = bacc.Bacc("TRN2", target_bir_lowering=False, debug=False,
                       num_devices=N_CORES)
        _emit(nc)
        nc.compile()
        _CACHE["nc"] = nc
    return _CACHE["nc"]


def _bf16(a):
    return np.ascontiguousarray(a.astype(ml_dtypes.bfloat16))


def kernel(x, qkv_w, qkv_b, out_w, out_b):
    x = np.asarray(x, dtype=np.float32)
    qkv_w = np.asarray(qkv_w, dtype=np.float32)
    qkv_b = np.asarray(qkv_b, dtype=np.float32)
    out_w = np.asarray(out_w, dtype=np.float32)
    out_b = np.asarray(out_b, dtype=np.float32)

    nc = _build()
    scale = HD ** -0.5
    in_maps = []
    for c in range(N_CORES):
        b, half = c // 2, c % 2
        wq = qkv_w[:, half * FQK:(half + 1) * FQK] * scale
        wk = qkv_w[:, D + half * FQK:D + (half + 1) * FQK]
        wv = qkv_w[:, 2 * D + half * FQK:2 * D + (half + 1) * FQK]
        bq = qkv_b[half * FQK:(half + 1) * FQK] * scale
        bk = qkv_b[D + half * FQK:D + (half + 1) * FQK]
        bv = qkv_b[2 * D + half * FQK:2 * D + (half + 1) * FQK]
        # pair-interleaved qk weights: [q0,k0,q1,k1,q2,k2] blocks of 128
        wqk = np.empty((D, 6 * 128), dtype=np.float32)
        bqk = np.empty((6, 128), dtype=np.float32)
        for p in range(3):
            wqk[:, (2 * p) * 128:(2 * p + 1) * 128] = \
                wq[:, p * 128:(p + 1) * 128]
            wqk[:, (2 * p + 1) * 128:(2 * p + 2) * 128] = \
                wk[:, p * 128:(p + 1) * 128]
            bqk[2 * p] = bq[p * 128:(p + 1) * 128]
            bqk[2 * p + 1] = bk[p * 128:(p + 1) * 128]
        in_maps.append({
            "xt": _bf16(x[b].T),
            "wqk": _bf16(wqk),
            "wv": _bf16(wv),
            "bqk": np.ascontiguousarray(bqk.T, dtype=np.float32),
            "vb": np.ascontiguousarray(
                np.broadcast_to(bv, (128, FQK)), dtype=np.float32),
            "wout": _bf16(out_w[half * FQK:(half + 1) * FQK, :]),
        })

    res = run_bass_kernel_spmd(nc, in_maps, list(range(N_CORES)), trace=TRACE)
    parts = [np.asarray(res.results[c]["out"], dtype=np.float32)
             for c in range(N_CORES)]
    out = np.empty((B, S, D), dtype=np.float32)
    for b in range(B):
        out[b] = parts[2 * b] + parts[2 * b + 1] + out_b
    if TRACE:
        kernel.last_results = res
    return out


# revision 42
# speedup vs baseline: 1.2296x; 1.0025x over previous
"""Causal self-attention (B=4, S=2048, D=768, H=12) on 8 trn2 NeuronCores.

Sharding (Megatron-style): DP over the 4 batches x TP=2 over heads.
Core c handles batch c//2 with heads (c%2)*6 .. +6: qkv_proj column-parallel,
out_proj row-parallel; the TP pair's partial outputs are summed on the host.

Per-core kernel, all matmul operands bf16 (PSUM accumulation fp32):
  - qkT = (x @ Wqk)^T in [feat(part), s] layout, computed in 512-col quanta;
    bias applied on the PSUM->SBUF copy by DVE (tensor_scalar) so the ACT
    engine is reserved for exp. V in natural [s, feat] layout with a ones
    column per head (V') so PV also yields the softmax denominator.
  - attention per (head, 512-q-chunk): S^T tile = K_tile @ Q_chunk
    (contraction 64), exp on ACT batched 2 k-tiles per ACTIVATE -> pt bf16;
    causal via narrowed matmuls + per-diagonal-block mask mult (GPSIMD);
    PV as O[q(part),65] += pt_slice^T(stationary) @ V'(moving, N=65).
    PV flush is deferred TWO k-groups so exp results are always ready when
    the PE reaches them (no exp-latency stall), keeping ACT the pacer.
  - the engine-work gap between ACT (exp) and PE inside attention is filled
    from a queue of ~1us PE work quanta (remaining qkT columns, V tiles,
    O-transposes, out-proj tiles), paced by a static work-debt counter.
  - normalize per (h,qc): per-partition reciprocal of the ones column +
    tensor_scalar -> O_norm [q, feat] bf16; PE-transpose back to oT
    [feat(part), q]; out_partial = O @ Wout_slice, stored bf16 (host sums
    the TP pair in fp32).
"""
import numpy as np
import ml_dtypes
import concourse.bass as bass
import concourse.mybir as mybir
import concourse.tile as tile
from concourse import bacc
from concourse.bass_utils import run_bass_kernel_spmd
from concourse.masks import make_identity

B, S, D = 4, 2048, 768
H, HD = 12, 64
N_CORES = 8
HPC = H // 2          # heads per core = 6
FQK = HPC * HD        # 384 features per core for each of q,k,v
F32 = mybir.dt.float32
BF16 = mybir.dt.bfloat16

N_ST = S // 128       # 16 s tiles
N_QC = S // 512       # 4 q chunks
N_DT = D // 128       # 6 d_model tiles
WARMUP_MM = 24
PE_CY = 0.4167        # ns/row, warm

TRACE = False
_CACHE = {}
PHASE_MARKS = []      # (phase_name, first_inst_id)


def _mark(nc, name):
    PHASE_MARKS.append((name, nc.next_id()))


def _emit(nc):
    xt_d = nc.dram_tensor("xt", [D, S], BF16, kind="ExternalInput").ap()
    wqk_d = nc.dram_tensor("wqk", [D, 6 * 128], BF16, kind="ExternalInput").ap()
    wv_d = nc.dram_tensor("wv", [D, FQK], BF16, kind="ExternalInput").ap()
    bqk_d = nc.dram_tensor("bqk", [128, 6], F32, kind="ExternalInput").ap()
    vb_d = nc.dram_tensor("vb", [128, FQK], F32, kind="ExternalInput").ap()
    wout_d = nc.dram_tensor("wout", [FQK, D], BF16, kind="ExternalInput").ap()
    out_d = nc.dram_tensor("out", [S, D], BF16, kind="ExternalOutput").ap()

    with tile.TileContext(nc) as tc:
        with tc.tile_pool(name="const", bufs=1) as pc, \
             tc.tile_pool(name="xTp", bufs=1) as pxt, \
             tc.tile_pool(name="qkTp", bufs=1) as pqk, \
             tc.tile_pool(name="vnp", bufs=1) as pvn, \
             tc.tile_pool(name="wstr", bufs=3) as pw, \
             tc.tile_pool(name="ptp", bufs=6) as ppt, \
             tc.tile_pool(name="onp", bufs=2) as pon, \
             tc.tile_pool(name="rp", bufs=2) as prp, \
             tc.tile_pool(name="oTp", bufs=1) as pot, \
             tc.tile_pool(name="outp", bufs=2) as pout, \
             tc.tile_pool(name="ps", bufs=2, space="PSUM") as pp, \
             tc.tile_pool(name="psv", bufs=2, space="PSUM") as ppv, \
             tc.tile_pool(name="psd", bufs=2, space="PSUM") as ppd:

            # ---- constants ----
            warm = pc.tile([128, 128], BF16)
            nc.vector.memset(warm[:], 0.25)
            identity = pc.tile([128, 128], BF16)
            make_identity(nc, identity)
            mask = pc.tile([128, 128], BF16)
            nc.gpsimd.memset(mask[:], 1.0)
            nc.gpsimd.affine_select(
                out=mask[:], in_=mask[:], compare_op=mybir.AluOpType.is_ge,
                fill=0.0, base=0, channel_multiplier=-1, pattern=[[1, 128]])

            bqk_sb = pc.tile([128, 6], F32)
            vb_sb = pc.tile([128, FQK], F32)
            wv_sb = pc.tile([128, N_DT, FQK], BF16)
            wout_sb = pc.tile([128, FQK // 128, D], BF16)

            xT = pxt.tile([128, N_DT, S], BF16)
            qkT = pqk.tile([128, 6, S], BF16)       # ft: q0,k0,q1,k1,q2,k2
            vn = pvn.tile([128, N_ST, HPC, HD + 1], BF16)
            oT = pot.tile([128, FQK // 128, S], BF16)

            _mark(nc, "A:load")
            # PE warmup: ramp the pstate + cover initial DMA latency
            for i in range(WARMUP_MM):
                ps_wm = ppd.tile([128, 512], F32, tag="d", name="ps_wm")
                nc.tensor.matmul(ps_wm[:, 0:128], warm[:], warm[:],
                                 start=True, stop=True)

            w_ts = {}

            def load_wpair(p):
                w_t = pw.tile([128, N_DT, 256], BF16, tag="w",
                              name=f"w_t{p}")
                nc.sync.dma_start(
                    w_t[:],
                    wqk_d[:, p * 256:(p + 1) * 256].rearrange(
                        "(t p) f -> p t f", p=128))
                w_ts[p] = w_t

            nc.sync.dma_start(bqk_sb[:], bqk_d[:])
            nc.sync.dma_start(
                xT[:, 0:3, 0:512],
                xt_d[0:384, 0:512].rearrange("(t p) s -> p t s", p=128))
            load_wpair(0)
            nc.sync.dma_start(
                xT[:, 3:6, 0:512],
                xt_d[384:768, 0:512].rearrange("(t p) s -> p t s", p=128))
            nc.sync.dma_start(
                wv_sb[:], wv_d.rearrange("(t p) f -> p t f", p=128))
            nc.sync.dma_start(vb_sb[:], vb_d[:])
            nc.sync.dma_start(
                xT[:, :, 512:1024],
                xt_d[:, 512:1024].rearrange("(t p) s -> p t s", p=128))
            load_wpair(1)
            nc.sync.dma_start(
                xT[:, :, 1024:1536],
                xt_d[:, 1024:1536].rearrange("(t p) s -> p t s", p=128))
            load_wpair(2)
            nc.sync.dma_start(
                xT[:, :, 1536:2048],
                xt_d[:, 1536:2048].rearrange("(t p) s -> p t s", p=128))
            nc.sync.dma_start(
                wout_sb[:], wout_d.rearrange("(t p) o -> p t o", p=128))

            nc.vector.memset(vn[:, :, :, HD:HD + 1], 1.0)
            vb_h = vb_sb.rearrange("p (h d) -> p h d", d=HD)

            # ---- work quanta ----
            def emit_v(st):
                ps_v = ppd.tile([128, 512], F32, tag="d", name="ps_v")
                for dc in range(N_DT):
                    nc.tensor.matmul(
                        ps_v[:, 0:FQK],
                        xT[:, dc, st * 128:(st + 1) * 128],
                        wv_sb[:, dc, :],
                        start=(dc == 0), stop=(dc == N_DT - 1))
                nc.vector.tensor_tensor(
                    vn[:, st, :, 0:HD],
                    ps_v[:, 0:FQK].rearrange("p (h d) -> p h d", d=HD),
                    vb_h, mybir.AluOpType.add)

            def emit_qk(ft, ch, pool_tag="d"):
                # qkT[:, ft, ch*512:(ch+1)*512] = (x @ w_ft)^T + b_ft
                p, fip = ft // 2, ft % 2
                w_t = w_ts[p]
                pool = ppd if pool_tag == "d" else pp
                ps_qk = pool.tile([128, 512], F32, tag=pool_tag,
                                  name="ps_qk")
                for dc in range(N_DT):
                    nc.tensor.matmul(
                        ps_qk[:],
                        w_t[:, dc, fip * 128:(fip + 1) * 128],
                        xT[:, dc, ch * 512:(ch + 1) * 512],
                        start=(dc == 0), stop=(dc == N_DT - 1))
                nc.vector.tensor_scalar(
                    qkT[:, ft, ch * 512:(ch + 1) * 512], ps_qk[:],
                    bqk_sb[:, ft:ft + 1], None, mybir.AluOpType.add)

            def transpose_o(qc, qt):
                onorm = onorms[qc]
                for k in range(FQK // 128):
                    tp = ppd.tile([128, 128], BF16, tag="d", name="tp")
                    nc.tensor.transpose(
                        tp[:], onorm[:, qt, k * 128:(k + 1) * 128],
                        identity[:])
                    nc.vector.tensor_copy(
                        oT[:, k, (qc * 4 + qt) * 128:(qc * 4 + qt + 1) * 128],
                        tp[:])

            def emit_proj(st):
                o_sb = pout.tile([128, D], BF16, tag="o_sb", name="o_sb")
                for oc in range(2):
                    ps_d = ppd.tile([128, 384], F32, tag="d", name="ps_d")
                    for ht in range(FQK // 128):
                        nc.tensor.matmul(
                            ps_d[:],
                            oT[:, ht, st * 128:(st + 1) * 128],
                            wout_sb[:, ht, oc * 384:(oc + 1) * 384],
                            start=(ht == 0),
                            stop=(ht == FQK // 128 - 1))
                    nc.vector.tensor_copy(
                        o_sb[:, oc * 384:(oc + 1) * 384], ps_d[:])
                    nc.sync.dma_start(
                        out_d[st * 128:(st + 1) * 128,
                              oc * 384:(oc + 1) * 384],
                        o_sb[:, oc * 384:(oc + 1) * 384])

            # ---- filler queue: (kind, args, pe_ns) popped by work debt ----
            filler = []

            def filler_run(item):
                kind, args = item[0], item[1]
                if kind == "v":
                    emit_v(args)
                elif kind == "qk":
                    emit_qk(*args)
                elif kind == "tproj":
                    qc, qt = args
                    transpose_o(qc, qt)
                    emit_proj(qc * 4 + qt)

            QK_NS = 6 * 512 * PE_CY
            V_NS = 6 * 384 * PE_CY
            TPROJ_NS = (3 * 128 + 6 * 384) * PE_CY

            vdone = {st: False for st in range(16)}
            qkdone = set()

            def note_done(item):
                if item[0] == "v":
                    vdone[item[1]] = True
                elif item[0] == "qk":
                    qkdone.add(item[1])

            state = {"debt": 0.0}

            def pump(ns):
                # clamp: deep-negative debt would starve the queue for whole
                # q-chunks; large-positive would dump bursts that starve ACT
                state["debt"] = min(max(state["debt"] + ns, -2000.0), 2400.0)
                while filler and state["debt"] >= 0.6 * filler[0][2]:
                    item = filler.pop(0)
                    state["debt"] -= item[2]
                    filler_run(item)
                    note_done(item)

            def force(need_v_st, need_qk):
                # selectively pull prerequisite items out of the filler
                need = [("v", st) for st in need_v_st if not vdone[st]]
                need += [("qk", q) for q in need_qk if q not in qkdone]
                for key in need:
                    for i, item in enumerate(filler):
                        if (item[0], item[1]) == key:
                            filler.pop(i)
                            filler_run(item)
                            note_done(item)
                            break

            # ---- attention ----
            pend = []   # deferred PV-flush closures, depth 2
            onorms = {}

            def attn_job(h, qc, inject=(), tail_mode=False):
                po = (h % 2) * 64
                qft = 2 * (h // 2)
                kft = qft + 1
                n_kt = 4 * (qc + 1)
                inject = list(inject)
                pv = ppv.tile([128, 4, HD + 1], F32, tag="v", name="pv")

                def mk_flush(prev_grp, last):
                    def flush():
                        ktg, offs, pt = prev_grp
                        for j in range(2):
                            kt = ktg + j
                            q_off = offs[j]
                            for qt in range(4):
                                q_lo = qt * 128
                                if q_lo < q_off:
                                    continue
                                # start only on the job's first matmul: on
                                # HW, start=True clears has_written for the
                                # WHOLE bank, which would wipe accumulation
                                # continuity of the other q-tiles' groups
                                nc.tensor.matmul(
                                    pv[:, qt, :],
                                    pt[:, j * 512 + q_lo:j * 512 + q_lo + 128],
                                    vn[:, kt, h, :],
                                    start=(kt == 0 and qt == 0),
                                    stop=(kt == 4 * qc + qt))
                        if tail_mode:
                            # drain each q-tile the moment its diagonal
                            # k-tile lands: normalize + transpose + out-proj,
                            # interleaved across the group's two q-tiles so
                            # PE transposes overlap the other tile's copies
                            qts = [ktg + j - 4 * qc for j in range(2)
                                   if 0 <= ktg + j - 4 * qc < 4]
                            for qt in qts:
                                normalize_qt(h, qc, pv, qt)
                            for qt in qts:
                                transpose_o(qc, qt)
                            for qt in qts:
                                emit_proj(qc * 4 + qt)
                        elif last:
                            job_done(h, qc, pv)
                    return flush

                for ktg in range(0, n_kt, 2):
                    ps_s = pp.tile([128, 1024], F32, tag="s", name="ps_s")
                    offs = []
                    for j in range(2):
                        kt = ktg + j
                        q_off = max(0, kt * 128 - qc * 512)
                        offs.append(q_off)
                        nc.tensor.matmul(
                            ps_s[:, j * 512 + q_off:(j + 1) * 512],
                            qkT[po:po + 64, kft, kt * 128:(kt + 1) * 128],
                            qkT[po:po + 64, qft,
                                qc * 512 + q_off:(qc + 1) * 512],
                            start=True, stop=True)
                    pt = ppt.tile([128, 1024], BF16, tag="pt", name="pt")
                    nc.scalar.activation(
                        pt[:, offs[0]:], ps_s[:, offs[0]:],
                        mybir.ActivationFunctionType.Exp)
                    for j in range(2):
                        kt = ktg + j
                        if kt >= 4 * qc:
                            qt_d = kt - 4 * qc
                            sl = slice(j * 512 + qt_d * 128,
                                       j * 512 + qt_d * 128 + 128)
                            nc.gpsimd.tensor_tensor(
                                pt[:, sl], pt[:, sl], mask[:],
                                mybir.AluOpType.mult)
                    for _ in range(2):
                        if inject:
                            item = inject.pop(0)
                            filler_run(item)
                            note_done(item)
                            state["debt"] -= item[2]
                    # pump BEFORE the deferred-PV pop: filler work is always
                    # ready, while PV flushes may still wait on exp — behind
                    # them in the in-order queue, ready work would stall
                    grp_rows = (512 - offs[0]) + (512 - offs[1])
                    pv_rows = sum(65 for j in range(2) for qt in range(4)
                                  if qt * 128 >= offs[j])
                    act_ns = (1024 - offs[0]) * 0.833 + 217
                    pump(act_ns - (grp_rows + pv_rows) * PE_CY)
                    while len(pend) >= (1 if tail_mode else 2):
                        pend.pop(0)()
                    pend.append(mk_flush((ktg, offs, pt),
                                         ktg == n_kt - 2))
                for item in inject:
                    filler_run(item)
                    note_done(item)
                    state["debt"] -= item[2]

            def normalize_qt(h, qc, pv, qt):
                onorm = onorms[qc]
                r1 = prp.tile([128, 1], F32, tag="r1", name="r1")
                nc.vector.reciprocal(r1[:], pv[:, qt, HD:HD + 1])
                nc.vector.tensor_scalar(
                    onorm[:, qt, h * HD:(h + 1) * HD],
                    pv[:, qt, 0:HD], r1[:], None, mybir.AluOpType.mult)

            def job_done(h, qc, pv):
                # job h's accumulation is complete: normalize into onorm
                if qc not in onorms:
                    onorms[qc] = pon.tile([128, 4, FQK], BF16, tag="on",
                                          name=f"onorm{qc}")
                onorm = onorms[qc]
                r = prp.tile([128, 4], F32, tag="r", name="r")
                nc.vector.reciprocal(r[:], pv[:, :, HD])
                for qt in range(4):
                    nc.vector.tensor_scalar(
                        onorm[:, qt, h * HD:(h + 1) * HD],
                        pv[:, qt, 0:HD], r[:, qt:qt + 1], None,
                        mybir.AluOpType.mult)
                if h == HPC - 1 and qc < 2:
                    # qc2's tprojs are injected explicitly in qc3; qc3's
                    # are handled by the tail-mode per-qt drain
                    for qt in range(4):
                        filler.append(("tproj", (qc, qt), TPROJ_NS))

            _mark(nc, "B:start")
            emit_qk(0, 0, pool_tag="s")
            emit_qk(1, 0, pool_tag="s")

            # all remaining qkT columns / V tiles as per-job injected quanta,
            # each placed with slack before its deadline (see flush timing)
            QK = lambda ft, ch: ("qk", (ft, ch), QK_NS)
            VQ = lambda st: ("v", st, V_NS)
            inject_map = {
                (0, 0): [VQ(0), VQ(1), VQ(2), VQ(3)],
                (0, 1): [QK(2, 0), QK(3, 0)],
                (0, 2): [QK(4, 0)],
                (0, 3): [QK(5, 0)],
                (0, 4): [QK(0, 1), QK(1, 1)],
                (0, 5): [VQ(4), VQ(5), VQ(6), VQ(7)],
                (1, 0): [QK(2, 1), QK(3, 1)],
                (1, 1): [QK(4, 1), QK(5, 1)],
                (1, 2): [QK(0, 2), QK(1, 2)],
                (1, 4): [QK(2, 2), QK(3, 2)],
                # qc1 is PE-bound, qc2 ACT-bound: V8-11 (first read at
                # job(1,2)) and qk ch2 for heads 4+ fit qc2's slack
                (2, 0): [QK(0, 3), QK(1, 3),
                         VQ(8), VQ(9), VQ(10), VQ(11)],
                (2, 1): [QK(4, 2), QK(5, 2)],
                # qc3 is ACT-bound with PE slack; push deadline-permitting
                # work there (V12-15 first read at job(1,3), qk ch3 for
                # heads 2+ read at jobs (2..5,3))
                (3, 0): [VQ(12), VQ(13), VQ(14), VQ(15),
                         QK(2, 3), QK(3, 3)],
                (3, 1): [QK(4, 3), QK(5, 3)],
                # qc2's transposes+projections as explicit qc3 injects (the
                # debt pump may defer them past the last exp otherwise)
                (3, 2): [("tproj", (2, 0), TPROJ_NS),
                         ("tproj", (2, 1), TPROJ_NS)],
                (3, 3): [("tproj", (2, 2), TPROJ_NS),
                         ("tproj", (2, 3), TPROJ_NS)],
            }
            for qc in range(N_QC):
                _mark(nc, f"C:attn qc={qc}")
                # onorm buffers rotate with depth 2: qc's normalize writes
                # reuse qc-2's buffer, so qc-2's transposes must be emitted
                for item in [f for f in filler
                             if f[0] == "tproj" and f[1][0] == qc - 2]:
                    filler.remove(item)
                    filler_run(item)
                for h in range(HPC):
                    need_qk = [(2 * (h // 2), ch) for ch in range(qc + 1)]
                    need_qk += [(2 * (h // 2) + 1, ch) for ch in range(qc + 1)]
                    need_qk = [q for q in need_qk if q[1] > 0]
                    # vn tiles are first read by the previous job's deferred
                    # PV flushes, so the requirement lags one job
                    v_hi = 4 * qc + 4 if h >= 1 else 4 * qc
                    force([st for st in range(4, v_hi)], need_qk)
                    attn_job(h, qc, inject=inject_map.get((qc, h), ()),
                             tail_mode=(qc == N_QC - 1 and h == HPC - 1))

            _mark(nc, "D:tail")
            while pend:
                pend.pop(0)()
            while filler:
                item = filler.pop(0)
                filler_run(item)


def _build():
    if "nc" not in _CACHE:
        nc = bacc.Bacc("TRN2", target_bir_lowering=False, debug=False,
                       num_devices=N_CORES)
        _emit(nc)
        nc.compile()
        _CACHE["nc"] = nc
    return _CACHE["nc"]


def _bf16(a):
    return np.ascontiguousarray(a.astype(ml_dtypes.bfloat16))


def kernel(x, qkv_w, qkv_b, out_w, out_b):
    x = np.asarray(x, dtype=np.float32)
    qkv_w = np.asarray(qkv_w, dtype=np.float32)
    qkv_b = np.asarray(qkv_b, dtype=np.float32)
    out_w = np.asarray(out_w, dtype=np.float32)
    out_b = np.asarray(out_b, dtype=np.float32)

    nc = _build()
    scale = HD ** -0.5
    in_maps = []
    for c in range(N_CORES):
        b, half = c // 2, c % 2
        wq = qkv_w[:, half * FQK:(half + 1) * FQK] * scale
        wk = qkv_w[:, D + half * FQK:D + (half + 1) * FQK]
        wv = qkv_w[:, 2 * D + half * FQK:2 * D + (half + 1) * FQK]
        bq = qkv_b[half * FQK:(half + 1) * FQK] * scale
        bk = qkv_b[D + half * FQK:D + (half + 1) * FQK]
        bv = qkv_b[2 * D + half * FQK:2 * D + (half + 1) * FQK]
        # pair-interleaved qk weights: [q0,k0,q1,k1,q2,k2] blocks of 128
        wqk = np.empty((D, 6 * 128), dtype=np.float32)
        bqk = np.empty((6, 128), dtype=np.float32)
        for p in range(3):
            wqk[:, (2 * p) * 128:(2 * p + 1) * 128] = \
                wq[:, p * 128:(p + 1) * 128]
            wqk[:, (2 * p + 1) * 128:(2 * p + 2) * 128] = \
                wk[:, p * 128:(p + 1) * 128]
            bqk[2 * p] = bq[p * 128:(p + 1) * 128]
            bqk[2 * p + 1] = bk[p * 128:(p + 1) * 128]
        in_maps.append({
            "xt": _bf16(x[b].T),
            "wqk": _bf16(wqk),
            "wv": _bf16(wv),
            "bqk": np.ascontiguousarray(bqk.T, dtype=np.float32),
            "vb": np.ascontiguousarray(
                np.broadcast_to(bv, (128, FQK)), dtype=np.float32),
            "wout": _bf16(out_w[half * FQK:(half + 1) * FQK, :]),
        })

    res = run_bass_kernel_spmd(nc, in_maps, list(range(N_CORES)), trace=TRACE)
    parts = [np.asarray(res.results[c]["out"], dtype=np.float32)
             for c in range(N_CORES)]
    out = np.empty((B, S, D), dtype=np.float32)
    for b in range(B):
        out[b] = parts[2 * b] + parts[2 * b + 1] + out_b
    if TRACE:
        kernel.last_results = res
    return out
